# revision 35
# baseline (speedup 1.0000x reference)
"""Trainium2 Bass kernel for a 2D DWT (depthwise 8x8 conv, stride 2).

Reference computes a depthwise conv of x [16, 64, 256, 256] with 4 subband
filters that are outer products of an 8-tap low/high pair -> separable:
apply the (low|high) banded filter matrix along H via one matmul pass,
then along W via a second pass.  Output [16, 256, 125, 125] with channel
order [ll(64) | lh(64) | hl(64) | hh(64)].

Design notes (from trace iteration; baseline 158.7us -> ~120us):
- fp16 matmul operands (x cast on host): LDWEIGHTS pipelines fully behind
  back-to-back fp16 matmuls even at N=64 (measured 28ns issue spacing).
- Output stored as fp16 (host casts to f32): halves store traffic (fp32
  stores put the per-core DMA floor at ~138us).
- BOTH passes are "banded": with x stored as row-halves (partition p =
  rows p and p+128), BM[0:128] columns are nonzero only for y 0..63 and
  BM[128:256] only for y 61..124, so each pass is 8 matmuls of N=64
  (512 cycles) instead of 4 of N=256 (1024); the y 61..63 overlap
  accumulates at identical PSUM addresses inside one group.  One [128,512]
  fp16 band-matrix constant serves both passes.
- STEADY STATE IS AT THE PER-CORE HBM WALL: all 16 SDMA engines measure
  94-100% busy mid-run at ~330-350 GB/s mixed (HBM-per-NC limit is ~358).
  Per-pair DMA (518KB) = ~1.5us; copy engines ~1.2us; so only startup
  (~8us fixed framework preamble + ~2us fill), the compute-paced store
  tail (~15us), and residual stalls are improvable.  Kernel measures
  ~120us at full clock; beware a chip power state that intermittently
  runs ALL engines at 5/6 clock (MM dur 234 vs 200ns) inflating runs to
  ~133us; compare runs only at equal clock (MM-dur mean).
- PSUM layout (8 banks): aps pair-tiles [128,1024]f32 x3 bufs (6 banks)
  + per-image bps [128,512] x2 (2 banks).  aps NEEDS 3 bufs: with 2, the
  chain A(k) -> CAST(k) on DVE -> (slow ~1.2us DVE->PE sem propagation)
  -> A(k+2) binds the cadence.  ACT->X propagation is fast (~26ns).
- LAG=2 software pipeline: B(k) is emitted after A(k+2) so the pair-CAST
  (vector, 1024 cols, ~1.21us) fully hides behind 4 intervening matmul
  groups.  Per-image bt-copies on scalar (ACT is faster from PSUM:
  (172+FD)/1.2GHz vs DVE (120+FD)/0.96GHz).
- Loads (gpsimd queue) and stores (sync queue) must be on separate queues
  (head-of-line blocking otherwise).  Load prefetch = 6 tiles; fewer (4)
  starves PE at load boundaries (+3us), more doesn't refill (prefetch
  cushion only ever shrinks: loads release 1-per-tile-consumed).
- Stores MUST cover all 128 partitions: a 125-partition store AP
  degenerates onto 5 of 16 SDMA engines (measured 207us total).
- First x tile is loaded as 4 quarter-DMAs (Tile deps are per-range) so
  the first matmul starts ~10us instead of 13.6; bm const load on the
  idle sync queue.
- Host pre-transposes x to [b, c-group, p, (c2, half, w)] so each load is
  one plain 2D DMA with 4KB contiguous per partition.

Sharding: pure data parallel over batch, 2 batches (128 images) per core.
"""

import numpy as np

B, C, H, W = 16, 64, 256, 256
HP = WP = 125
N_CORES = 8
B_SH = B // N_CORES  # 2 batches per core
GRP = 4  # images per output store (4 KB per partition per store)
BANDED_B = True  # banded pass-B moving slices (N=64 x8 vs N=256 x4)
LGRP = 2  # images per input load DMA (finer release quanta than 4)
LAG = 3  # software-pipeline depth in pairs (B(k) emitted after A(k+LAG))
XBUFS = 8  # input prefetch tiles (end-of-run backlog = XBUFS*LGRP images)
ABUFS = 8  # asb tiles (must cover LAG + cast-in-flight)
BBUFS = 8  # bt store-staging tiles (covers store DMA + HBM receipt latency)

_LOW = np.array(
    [0.1629, 0.5055, 0.4464, -0.0198, -0.1323, 0.0218, 0.0233, -0.0075],
    dtype=np.float32,
)
_HIGH = np.array(
    [-0.0075, -0.0233, 0.0218, 0.1323, -0.0198, -0.4464, 0.5055, -0.1629],
    dtype=np.float32,
)


def _band_matrix() -> np.ndarray:
    """BM[h, f*128 + y] = filt_f[h - 2y] for 0 <= h-2y < 8.

    Columns 125:128 and 253:256 are zero padding so each filter block is
    128 wide (full-width stationary operands, moving free dim 256).
    """
    bm = np.zeros((256, 256), dtype=np.float32)
    for f, filt in enumerate((_LOW, _HIGH)):
        for y in range(125):
            bm[2 * y : 2 * y + 8, f * 128 + y] = filt
    return bm


def _band_consts() -> np.ndarray:
    """[128, 512] fp16: BM[0:128] | BM[128:256] (used by both passes)."""
    bm = _band_matrix()
    return np.concatenate([bm[0:128], bm[128:256]], axis=1).astype(np.float16)


_CACHE = {}


def _build_bass():
    import concourse.bacc as bacc
    import concourse.mybir as mybir
    from concourse.tile import TileContext

    f32 = mybir.dt.float32
    f16 = mybir.dt.float16

    nc = bacc.Bacc("TRN2")
    # x pre-transposed on host to [b, c-group, p, (c2 r w)]: partition p
    # holds rows 2p, 2p+1 of LGRP consecutive channel-images -> each load
    # is a plain 2D DMA with LGRP KB contiguous per partition.
    x_d = nc.dram_tensor(
        "x", [B_SH, C // LGRP, 128, LGRP * 512], f16, kind="ExternalInput"
    )
    bm_d = nc.dram_tensor("bmc", [128, 512], f16, kind="ExternalInput")
    # [b, c//GRP, hy(128), c%GRP, subband, wx]: each (b, c-group) is one
    # contiguous block with hy outermost.  hy runs to 128 (3 pad rows the
    # host strips): stores sourced from 128 SBUF partitions spread across
    # all 16 SDMA engines, while 125-partition stores land on only 5
    # (measured; partition count is what decides the spread).
    out_d = nc.dram_tensor(
        "out", [B_SH, C // GRP, 128, GRP, 4, WP], f16, kind="ExternalOutput"
    )

    with TileContext(nc) as tc:
        with (
            tc.tile_pool(name="const", bufs=1) as cpool,
            tc.tile_pool(name="xin", bufs=XBUFS) as xpool,
            tc.tile_pool(name="asb", bufs=ABUFS) as apool,
            tc.tile_pool(name="bsb", bufs=BBUFS) as bpool,
            tc.tile_pool(name="aps", bufs=3, space="PSUM") as apspool,
            tc.tile_pool(name="bps", bufs=2, space="PSUM") as bpspool,
        ):
            bma = cpool.tile([128, 512], f16, tag="bma")
            nc.sync.dma_start(out=bma[:], in_=bm_d[:])
            bm0 = bma[:, 0:256]
            bm1 = bma[:, 256:512]

            # Flat pair pipeline, software-pipelined by one pair: the PE
            # stream is A(k), B(k-1), A(k+1), B(k), ... so B never waits
            # for the CAST that feeds it (the CAST runs during the next
            # pair's A matmuls).  Two images per PSUM tile (one bank
            # each); one vector CAST and one scalar bt-copy per pair.
            n_pairs = (B_SH * C) // 2
            ppg = GRP // 2  # pairs per store group
            ppl = LGRP // 2  # pairs per load

            def emit_b(p, par=False):
                asb2, bt, j0, store = p
                # one single-bank PSUM tile + one copy per image: keeps
                # bps at 2 banks so aps can have 3 bufs (8 banks total),
                # breaking the A(k+2)-waits-CAST(k) critical cycle
                for i, h in enumerate((0, 512)):
                    bps1 = bpspool.tile([128, 512], f32, tag="bps")
                    _pass_b(nc, asb2, h, bps1, bm0, bm1)
                    src = bps1[:].rearrange("p (v g x) -> p v g x", v=2, g=2)
                    dst = bt[
                        :, (j0 + i) * 500 : (j0 + i) * 500 + 500
                    ].rearrange("p (v g x) -> p v g x", v=2, g=2)
                    # during the drain (par=True) the vector engine is idle
                    # after its last CAST: alternate copies across both
                    # engines so each flushed pair's copies run in parallel
                    if par and i == 1:
                        nc.vector.tensor_copy(dst, src[:, :, :, 0:125])
                    else:
                        nc.scalar.copy(dst, src[:, :, :, 0:125])
                if store is not None:
                    # must store all 128 partitions: a 125-partition AP
                    # degenerates to 5 SDMA engines (measured 207us)
                    nc.sync.dma_start(out=store, in_=bt[:])

            from collections import deque

            pending = deque()
            bt = xt = None
            for k in range(n_pairs):
                img0 = k * 2
                b, c = img0 // C, img0 % C
                if k % ppl == 0:
                    xt = xpool.tile([128, LGRP * 512], f16, tag="xt")
                    src = x_d[b, c // LGRP]
                    # half-split the very first tile so matmuls start
                    # sooner (tile deps are per-range), and issue it on
                    # the sync HWDGE queue: it is live ~1.5us before the
                    # gpsimd SWDGE path warms up, and carries no store
                    # traffic until ~21us
                    nq = 2 if k == 0 else 1
                    eng = nc.sync if k == 0 else nc.gpsimd
                    step = (LGRP * 512) // nq
                    for q in range(nq):
                        eng.dma_start(
                            out=xt[:, q * step : (q + 1) * step],
                            in_=src[:, q * step : (q + 1) * step],
                        )
                if k % ppg == 0:
                    bt = bpool.tile([128, GRP * 500], f16, tag="bt")
                aps2 = apspool.tile([128, 1024], f32, tag="aps")
                asb2 = apool.tile([128, 1024], f16, tag="asb")
                jp = (k % ppl) * 2
                for jj in (jp, jp + 1):
                    _pass_a(nc, xt, jj, aps2, (jj - jp) * 512, bm0, bm1)
                nc.vector.tensor_copy(asb2[:], aps2[:])
                store = None
                if k % ppg == ppg - 1:
                    store = out_d[b, c // GRP].rearrange("h c s w -> h (c s w)")
                pending.append((asb2, bt, (k % ppg) * 2, store))
                if len(pending) > LAG:
                    emit_b(pending.popleft())
            while pending:
                emit_b(pending.popleft(), par=True)
    nc.finalize()
    return nc


def _copy(eng, dst, src):
    if hasattr(eng, "tensor_copy"):
        eng.tensor_copy(dst, src)
    else:
        eng.copy(dst, src)


def _pass_a(nc, xt, jj, aps2, h, bm0, bm1):
    """A[w, f*128+hy] = sum_h x[h,w]*BM[h, f*128+hy], banded.

    xt partition p holds rows p (half 0) and p+128 (half 1), so the
    contraction is over raw rows: BM[0:128] columns are nonzero only for
    hy 0..63 and BM[128:256] only for hy 61..124.  The hy 61..63 overlap
    accumulates at identical PSUM addresses within one group (the group
    confined to this image's bank, cols [h, h+512)).
    """
    x0 = jj * 512
    n = 0
    for wc in range(2):
        for half in range(2):
            st = xt[:, x0 + half * 256 + wc * 128 : x0 + half * 256 + wc * 128 + 128]
            bm = bm0 if half == 0 else bm1
            c0 = 0 if half == 0 else 61
            for f in range(2):
                oc = h + wc * 256 + f * 128 + c0
                nc.tensor.matmul(
                    aps2[:, oc : oc + 64],
                    st,
                    bm[:, f * 128 + c0 : f * 128 + c0 + 64],
                    start=(n == 0),
                    stop=(n == 7),
                    skip_group_check=True,
                )
                n += 1


def _pass_b(nc, asb2, h, bps1, bm0, bm1):
    """B[hy, g*128+wx] = sum_w A[w, f*128+hy] * BM[w, g*128+wx], banded.

    BM[w 0..127] cols are nonzero only for wx 0..63; BM[w 128..255] only
    for wx 61..124; the wx 61..63 overlap accumulates in PSUM.
    Output for this image goes to its own single-bank tile, cols 0..511.
    """
    n = 0
    for fv in range(2):
        for wc in range(2):
            st = asb2[:, h + wc * 256 + fv * 128 : h + wc * 256 + fv * 128 + 128]
            for g in range(2):
                if wc == 0:
                    mv = bm0[:, g * 128 : g * 128 + 64]
                    oc = fv * 256 + g * 128
                else:
                    mv = bm1[:, g * 128 + 61 : g * 128 + 125]
                    oc = fv * 256 + g * 128 + 61
                nc.tensor.matmul(
                    bps1[:, oc : oc + 64],
                    st,
                    mv,
                    start=(n == 0),
                    stop=(n == 7),
                    skip_group_check=True,
                )
                n += 1


def kernel(x: np.ndarray, trace: bool = False):
    from concourse.bass_utils import run_bass_kernel_spmd

    x = np.asarray(x)
    assert x.shape == (B, C, H, W), x.shape
    # [b, c-group, p, c2, half, w]: partition p = rows p, p+128 per image
    x16 = np.ascontiguousarray(
        x.astype(np.float16)
        .reshape(B, C // LGRP, LGRP, 2, H // 2, W)
        .transpose(0, 1, 4, 2, 3, 5)
    )

    if "nc" not in _CACHE:
        _CACHE["nc"] = _build_bass()
    nc = _CACHE["nc"]

    bmc = _band_consts()
    in_maps = [
        {"x": x16[i * B_SH : (i + 1) * B_SH], "bmc": bmc} for i in range(N_CORES)
    ]
    res = run_bass_kernel_spmd(
        nc, in_maps, core_ids=list(range(N_CORES)), trace=trace
    )
    # [16, C//GRP, 128, GRP, 4, 125] (b, cg, hy+pad, cj, s, wx)
    #   -> strip 3 hy pad rows -> (b, s, cg, cj, hy, wx) -> [16, 256, 125, 125]
    raw = np.concatenate([r["out"] for r in res.results], axis=0)[:, :, :HP]
    out = (
        np.ascontiguousarray(raw.transpose(0, 4, 1, 3, 2, 5))
        .reshape(B, 4 * C, HP, WP)
        .astype(np.float32)
    )
    if trace:
        return out, res
    return out



# revision 37
# speedup vs baseline: 1.0760x; 1.0760x over previous
"""Trainium2 Bass kernel for a 2D DWT (depthwise 8x8 conv, stride 2).

Reference computes a depthwise conv of x [16, 64, 256, 256] with 4 subband
filters that are outer products of an 8-tap low/high pair -> separable:
apply the (low|high) banded filter matrix along H via one matmul pass,
then along W via a second pass.  Output [16, 256, 125, 125] with channel
order [ll(64) | lh(64) | hl(64) | hh(64)].

Design notes (from trace iteration; baseline 158.7us -> ~120us):
- fp16 matmul operands (x cast on host): LDWEIGHTS pipelines fully behind
  back-to-back fp16 matmuls even at N=64 (measured 28ns issue spacing).
- Output stored as fp16 (host casts to f32): halves store traffic (fp32
  stores put the per-core DMA floor at ~138us).
- BOTH passes are "banded": with x stored as row-halves (partition p =
  rows p and p+128), BM[0:128] columns are nonzero only for y 0..63 and
  BM[128:256] only for y 61..124, so each pass is 8 matmuls of N=64
  (512 cycles) instead of 4 of N=256 (1024); the y 61..63 overlap
  accumulates at identical PSUM addresses inside one group.  One [128,512]
  fp16 band-matrix constant serves both passes.
- STEADY STATE IS AT THE PER-CORE HBM WALL: all 16 SDMA engines measure
  94-100% busy mid-run at ~330-350 GB/s mixed (HBM-per-NC limit is ~358).
  Per-pair DMA (518KB) = ~1.5us; copy engines ~1.2us; so only startup
  (~8us fixed framework preamble + ~2us fill), the compute-paced store
  tail (~15us), and residual stalls are improvable.  Kernel measures
  ~120us at full clock; beware a chip power state that intermittently
  runs ALL engines at 5/6 clock (MM dur 234 vs 200ns) inflating runs to
  ~133us; compare runs only at equal clock (MM-dur mean).
- PSUM layout (8 banks): aps pair-tiles [128,1024]f32 x3 bufs (6 banks)
  + per-image bps [128,512] x2 (2 banks).  aps NEEDS 3 bufs: with 2, the
  chain A(k) -> CAST(k) on DVE -> (slow ~1.2us DVE->PE sem propagation)
  -> A(k+2) binds the cadence.  ACT->X propagation is fast (~26ns).
- LAG=2 software pipeline: B(k) is emitted after A(k+2) so the pair-CAST
  (vector, 1024 cols, ~1.21us) fully hides behind 4 intervening matmul
  groups.  Per-image bt-copies on scalar (ACT is faster from PSUM:
  (172+FD)/1.2GHz vs DVE (120+FD)/0.96GHz).
- Loads (gpsimd queue) and stores (sync queue) must be on separate queues
  (head-of-line blocking otherwise).  Load prefetch = 6 tiles; fewer (4)
  starves PE at load boundaries (+3us), more doesn't refill (prefetch
  cushion only ever shrinks: loads release 1-per-tile-consumed).
- Stores MUST cover all 128 partitions: a 125-partition store AP
  degenerates onto 5 of 16 SDMA engines (measured 207us total).
- First x tile is loaded as 4 quarter-DMAs (Tile deps are per-range) so
  the first matmul starts ~10us instead of 13.6; bm const load on the
  idle sync queue.
- Host pre-transposes x to [b, c-group, p, (c2, half, w)] so each load is
  one plain 2D DMA with 4KB contiguous per partition.

Sharding: pure data parallel over batch, 2 batches (128 images) per core.
"""

import numpy as np

B, C, H, W = 16, 64, 256, 256
HP = WP = 125
N_CORES = 8
B_SH = B // N_CORES  # 2 batches per core
GRP = 4  # images per output store (4 KB per partition per store)
BANDED_B = True  # banded pass-B moving slices (N=64 x8 vs N=256 x4)
LGRP = 2  # images per input load DMA (finer release quanta than 4)
LAG = 3  # software-pipeline depth in pairs (B(k) emitted after A(k+LAG))
XBUFS = 8  # input prefetch tiles (end-of-run backlog = XBUFS*LGRP images)
ABUFS = 8  # asb tiles (must cover LAG + cast-in-flight)
BBUFS = 8  # bt store-staging tiles (covers store DMA + HBM receipt latency)

_LOW = np.array(
    [0.1629, 0.5055, 0.4464, -0.0198, -0.1323, 0.0218, 0.0233, -0.0075],
    dtype=np.float32,
)
_HIGH = np.array(
    [-0.0075, -0.0233, 0.0218, 0.1323, -0.0198, -0.4464, 0.5055, -0.1629],
    dtype=np.float32,
)


def _band_matrix() -> np.ndarray:
    """BM[h, f*128 + y] = filt_f[h - 2y] for 0 <= h-2y < 8.

    Columns 125:128 and 253:256 are zero padding so each filter block is
    128 wide (full-width stationary operands, moving free dim 256).
    """
    bm = np.zeros((256, 256), dtype=np.float32)
    for f, filt in enumerate((_LOW, _HIGH)):
        for y in range(125):
            bm[2 * y : 2 * y + 8, f * 128 + y] = filt
    return bm


def _band_consts() -> np.ndarray:
    """[128, 512] fp16: BM[0:128] | BM[128:256] (used by both passes)."""
    bm = _band_matrix()
    return np.concatenate([bm[0:128], bm[128:256]], axis=1).astype(np.float16)


_CACHE = {}


def _build_bass():
    import concourse.bacc as bacc
    import concourse.mybir as mybir
    from concourse.tile import TileContext

    f32 = mybir.dt.float32
    f16 = mybir.dt.float16

    nc = bacc.Bacc("TRN2")
    # x pre-transposed on host to [b, c-group, p, (c2 r w)]: partition p
    # holds rows 2p, 2p+1 of LGRP consecutive channel-images -> each load
    # is a plain 2D DMA with LGRP KB contiguous per partition.
    x_d = nc.dram_tensor(
        "x", [B_SH, C // LGRP, 128, LGRP * 512], f16, kind="ExternalInput"
    )
    bm_d = nc.dram_tensor("bmc", [128, 512], f16, kind="ExternalInput")
    # [b, c//GRP, hy(128), c%GRP, subband, wx]: each (b, c-group) is one
    # contiguous block with hy outermost.  hy runs to 128 (3 pad rows the
    # host strips): stores sourced from 128 SBUF partitions spread across
    # all 16 SDMA engines, while 125-partition stores land on only 5
    # (measured; partition count is what decides the spread).
    out_d = nc.dram_tensor(
        "out", [B_SH, C // GRP, 128, GRP, 4, WP], f16, kind="ExternalOutput"
    )

    with TileContext(nc) as tc:
        with (
            tc.tile_pool(name="const", bufs=1) as cpool,
            tc.tile_pool(name="xin", bufs=XBUFS) as xpool,
            tc.tile_pool(name="asb", bufs=ABUFS) as apool,
            tc.tile_pool(name="bsb", bufs=BBUFS) as bpool,
            tc.tile_pool(name="aps", bufs=3, space="PSUM") as apspool,
            tc.tile_pool(name="bps", bufs=2, space="PSUM") as bpspool,
        ):
            bma = cpool.tile([128, 512], f16, tag="bma")
            nc.sync.dma_start(out=bma[:], in_=bm_d[:])
            bm0 = bma[:, 0:256]
            bm1 = bma[:, 256:512]

            # Flat pair pipeline, software-pipelined by one pair: the PE
            # stream is A(k), B(k-1), A(k+1), B(k), ... so B never waits
            # for the CAST that feeds it (the CAST runs during the next
            # pair's A matmuls).  Two images per PSUM tile (one bank
            # each); one vector CAST and one scalar bt-copy per pair.
            n_pairs = (B_SH * C) // 2
            ppg = GRP // 2  # pairs per store group
            ppl = LGRP // 2  # pairs per load

            def emit_b(p, par=False):
                asb2, bt, j0, store = p
                # one single-bank PSUM tile + one copy per image: keeps
                # bps at 2 banks so aps can have 3 bufs (8 banks total),
                # breaking the A(k+2)-waits-CAST(k) critical cycle
                for i, h in enumerate((0, 512)):
                    bps1 = bpspool.tile([128, 512], f32, tag="bps")
                    _pass_b(nc, asb2, h, bps1, bm0, bm1)
                    src = bps1[:].rearrange("p (v g x) -> p v g x", v=2, g=2)
                    dst = bt[
                        :, (j0 + i) * 500 : (j0 + i) * 500 + 500
                    ].rearrange("p (v g x) -> p v g x", v=2, g=2)
                    # during the drain (par=True) the vector engine is idle
                    # after its last CAST: alternate copies across both
                    # engines so each flushed pair's copies run in parallel
                    if par and i == 1:
                        nc.vector.tensor_copy(dst, src[:, :, :, 0:125])
                    else:
                        nc.scalar.copy(dst, src[:, :, :, 0:125])
                if store is not None:
                    # must store all 128 partitions: a 125-partition AP
                    # degenerates to 5 SDMA engines (measured 207us)
                    nc.sync.dma_start(out=store, in_=bt[:])

            from collections import deque

            pending = deque()
            bt = xt = None
            for k in range(n_pairs):
                img0 = k * 2
                b, c = img0 // C, img0 % C
                if k % ppl == 0:
                    xt = xpool.tile([128, LGRP * 512], f16, tag="xt")
                    src = x_d[b, c // LGRP]
                    # half-split the very first tile so matmuls start
                    # sooner (tile deps are per-range), and issue it on
                    # the sync HWDGE queue: it is live ~1.5us before the
                    # gpsimd SWDGE path warms up, and carries no store
                    # traffic until ~21us
                    nq = 2 if k == 0 else 1
                    eng = nc.sync if k == 0 else nc.gpsimd
                    step = (LGRP * 512) // nq
                    for q in range(nq):
                        eng.dma_start(
                            out=xt[:, q * step : (q + 1) * step],
                            in_=src[:, q * step : (q + 1) * step],
                        )
                if k % ppg == 0:
                    bt = bpool.tile([128, GRP * 500], f16, tag="bt")
                aps2 = apspool.tile([128, 1024], f32, tag="aps")
                asb2 = apool.tile([128, 1024], f16, tag="asb")
                jp = (k % ppl) * 2
                for jj in (jp, jp + 1):
                    _pass_a(nc, xt, jj, aps2, (jj - jp) * 512, bm0, bm1)
                nc.vector.tensor_copy(asb2[:], aps2[:])
                store = None
                if k % ppg == ppg - 1:
                    store = out_d[b, c // GRP].rearrange("h c s w -> h (c s w)")
                pending.append((asb2, bt, (k % ppg) * 2, store))
                if len(pending) > LAG:
                    emit_b(pending.popleft())
            while pending:
                emit_b(pending.popleft(), par=True)
    nc.finalize()
    return nc


def _copy(eng, dst, src):
    if hasattr(eng, "tensor_copy"):
        eng.tensor_copy(dst, src)
    else:
        eng.copy(dst, src)


def _pass_a(nc, xt, jj, aps2, h, bm0, bm1):
    """A[w, f*128+hy] = sum_h x[h,w]*BM[h, f*128+hy], banded.

    xt partition p holds rows p (half 0) and p+128 (half 1), so the
    contraction is over raw rows: BM[0:128] columns are nonzero only for
    hy 0..63 and BM[128:256] only for hy 61..124.  The hy 61..63 overlap
    accumulates at identical PSUM addresses within one group (the group
    confined to this image's bank, cols [h, h+512)).
    """
    x0 = jj * 512
    n = 0
    for wc in range(2):
        for half in range(2):
            st = xt[:, x0 + half * 256 + wc * 128 : x0 + half * 256 + wc * 128 + 128]
            bm = bm0 if half == 0 else bm1
            c0 = 0 if half == 0 else 61
            for f in range(2):
                oc = h + wc * 256 + f * 128 + c0
                nc.tensor.matmul(
                    aps2[:, oc : oc + 64],
                    st,
                    bm[:, f * 128 + c0 : f * 128 + c0 + 64],
                    start=(n == 0),
                    stop=(n == 7),
                    skip_group_check=True,
                )
                n += 1


def _pass_b(nc, asb2, h, bps1, bm0, bm1):
    """B[hy, g*128+wx] = sum_w A[w, f*128+hy] * BM[w, g*128+wx], banded.

    BM[w 0..127] cols are nonzero only for wx 0..63; BM[w 128..255] only
    for wx 61..124; the wx 61..63 overlap accumulates in PSUM.
    Output for this image goes to its own single-bank tile, cols 0..511.
    """
    n = 0
    for fv in range(2):
        for wc in range(2):
            st = asb2[:, h + wc * 256 + fv * 128 : h + wc * 256 + fv * 128 + 128]
            for g in range(2):
                if wc == 0:
                    mv = bm0[:, g * 128 : g * 128 + 64]
                    oc = fv * 256 + g * 128
                else:
                    mv = bm1[:, g * 128 + 61 : g * 128 + 125]
                    oc = fv * 256 + g * 128 + 61
                nc.tensor.matmul(
                    bps1[:, oc : oc + 64],
                    st,
                    mv,
                    start=(n == 0),
                    stop=(n == 7),
                    skip_group_check=True,
                )
                n += 1


def kernel(x: np.ndarray, trace: bool = False):
    from concourse.bass_utils import run_bass_kernel_spmd

    x = np.asarray(x)
    assert x.shape == (B, C, H, W), x.shape
    # [b, c-group, p, c2, half, w]: partition p = rows p, p+128 per image
    x16 = np.ascontiguousarray(
        x.astype(np.float16)
        .reshape(B, C // LGRP, LGRP, 2, H // 2, W)
        .transpose(0, 1, 4, 2, 3, 5)
    )

    if "nc" not in _CACHE:
        _CACHE["nc"] = _build_bass()
    nc = _CACHE["nc"]

    bmc = _band_consts()
    in_maps = [
        {"x": x16[i * B_SH : (i + 1) * B_SH], "bmc": bmc} for i in range(N_CORES)
    ]
    res = run_bass_kernel_spmd(
        nc, in_maps, core_ids=list(range(N_CORES)), trace=trace
    )
    # [16, C//GRP, 128, GRP, 4, 125] (b, cg, hy+pad, cj, s, wx)
    #   -> strip 3 hy pad rows -> (b, s, cg, cj, hy, wx) -> [16, 256, 125, 125]
    raw = np.concatenate([r["out"] for r in res.results], axis=0)[:, :, :HP]
    out = (
        np.ascontiguousarray(raw.transpose(0, 4, 1, 3, 2, 5))
        .reshape(B, 4 * C, HP, WP)
        .astype(np.float32)
    )
    if trace:
        return out, res
    return out

